# revision 72
# baseline (speedup 1.0000x reference)
"""Trainium2 Bass kernel for nn_LunaCausalAttention.

Sharding: 8 cores; core c handles batch b = c//4 and heads hs = 4*(c%4) .. hs+4.

v2 over the 82.7us baseline:
- q/k/pc/pq projections run in fp8e4m3 with DoubleRow perf mode (2x contract
  per pass, 0.5 cycles/row): weights are pre-scaled x16 on the host so they
  sit in fp8's normal range; the 1/16 factors fold into the host-side
  constants (rcb, beta2, biases) with no extra device ops. v and the output
  projection stay bf16 (fp8 there costs ~4e-2 end-to-end error).
- The backend compiler rejects programs that mix DoubleRow matmuls with
  sub-128-partition matmul operands, so the per-head G/G2 contractions use
  zero-padded stationaries (kTp / at_pad) and contract the full 128
  partitions; the zero half kills the other head's contribution. Padded
  tiles are persistent: their zero halves are memset once and never
  rewritten.
- K chunks are re-laid token-major by XBAR DMA transpose (SP queue, off the
  critical path) instead of PE transpose + DVE copy.
- Elementwise/copy ops are pinned to engines (GPSIMD has no PSUM port, so
  only SBUF-only work goes to Pool); gm/g2m masks are single wide DVE ops.
- S and T state copies merged into one [128,256] copy per attention call.
- The activation-table patch routes every act-func set to
  natural_log_exp_and_others so exactly one LoadActFuncSet is emitted.
- DMA order: fp8 x/weights first (halved bytes), xT fp8 split per-half so
  the first projection chain starts earlier; last chunk's output is split
  across the two head-pair passes to shorten the tail.
"""
import numpy as np

import concourse.bass as bass
import concourse.mybir as mybir
import concourse.tile as tile
from concourse import bacc
from concourse.masks import make_upper_triangular, make_identity
from concourse.bass_utils import run_bass_kernel_spmd

# static shapes
B, N, D, M, H, DH = 2, 1024, 1024, 64, 16, 64
C = 128                 # token chunk
NCH = N // C            # 8 chunks
NCORES = 8
HPC = 4                 # heads per core
E = HPC * DH            # 256 per-core head features
NF = D // 128           # 8 contraction tiles
NFP = NF // 2           # 4 DoubleRow pair tiles
BETA = float(np.log(2.0))
SCALE = DH ** -0.5
WS = 16.0               # host weight pre-scale for fp8
BETA2 = BETA * SCALE / (WS * WS)   # exp scale for pz (pc' ~ 16pc, pq' ~ 16pq)

F32 = mybir.dt.float32
F16 = mybir.dt.float16
BF16 = mybir.dt.bfloat16
F8 = mybir.dt.float8e4
ADT = BF16              # attention-core operand dtype
AF = mybir.ActivationFunctionType
ALU = mybir.AluOpType
PM = mybir.MatmulPerfMode


def build_bass(phase=3):
    nc = bacc.Bacc(None, target_bir_lowering=False)

    # ---- I/O ----
    # fp8 activation pair layout: [p, fp, t, n] = src[(2*fp+t)*128 + p, n]
    # fp8 weight pair layout: [p, et, fp, t, e] so each (et, fp) slice is a
    # contiguous [128, 2, 128] block (the verified DoubleRow AP shape).
    x8a_d = nc.dram_tensor("x8a", [128, NFP, 2, 512], F8, kind="ExternalInput")
    x8b_d = nc.dram_tensor("x8b", [128, NFP, 2, 512], F8, kind="ExternalInput")
    p8_d = nc.dram_tensor("p8", [128, NFP, 2, M], F8, kind="ExternalInput")
    w8q_d = nc.dram_tensor("w8q", [128, 2, NFP, 2, 128], F8,
                           kind="ExternalInput")
    w8k_d = nc.dram_tensor("w8k", [128, 2, NFP, 2, 128], F8,
                           kind="ExternalInput")
    w8pc_d = nc.dram_tensor("w8pc", [128, 2, NFP, 2, 128], F8,
                            kind="ExternalInput")
    w8pq_d = nc.dram_tensor("w8pq", [128, 2, NFP, 2, 128], F8,
                            kind="ExternalInput")
    xT_d = nc.dram_tensor("xT", [D, N], BF16, kind="ExternalInput")  # for v
    wv_d = nc.dram_tensor("wv", [D, E], BF16, kind="ExternalInput")
    wo_d = nc.dram_tensor("wo", [E, D], BF16, kind="ExternalInput")
    # packed f32 constants: bq 0:2 | bk 2:4 | bpc 4:6 | bpq 6:8 | rcc 8:16
    cst_d = nc.dram_tensor("cst", [128, 16], F32, kind="ExternalInput")
    bvr_d = nc.dram_tensor("bvr", [1, E], BF16, kind="ExternalInput")
    rcb_d = nc.dram_tensor("rcb", [128, N], BF16, kind="ExternalInput")
    ones2_d = nc.dram_tensor("ones2", [128, 2], BF16, kind="ExternalInput")
    out_d = nc.dram_tensor("outp", [N, D], BF16, kind="ExternalOutput")

    with tile.TileContext(nc) as tc:
        with (
            tc.tile_pool(name="singles", bufs=1) as singles,
            tc.tile_pool(name="work", bufs=5) as work,
            tc.tile_pool(name="obuf", bufs=4) as obuf,
            tc.tile_pool(name="psum", bufs=1, space="PSUM") as psum,
        ):
            # ---- constants (device-generated) ----
            triu2 = singles.tile([128, 2 * C], F32)     # two upper-tri copies
            make_upper_triangular(nc, triu2[:, 0:C], val=1.0, diag=True)
            make_upper_triangular(nc, triu2[:, C:2 * C], val=1.0, diag=True)
            identb = singles.tile([128, 128], ADT)
            make_identity(nc, identb)

            # ---- DMA, in compute-need order ----
            def load_small(shape, dt, dram, name):
                t = singles.tile(shape, dt, name=name)
                nc.sync.dma_start(out=t, in_=dram[:, :])
                return t

            def load_w8(name, dram):
                w = singles.tile([128, 2, NFP, 2, 128], F8, name=name)
                nc.sync.dma_start(out=w, in_=dram[:, :, :, :, :])
                return w

            xT_r = xT_d.rearrange("(f p) n -> p f n", p=128)
            x8a_sb = singles.tile([128, NFP, 2, 512], F8, name="x8a_sb")
            nc.sync.dma_start(out=x8a_sb, in_=x8a_d[:, :, :, :])
            w8pc_sb = load_w8("w8pc_sb", w8pc_d)
            cst = load_small([128, 16], F32, cst_d, "cst")
            bpc_sb = cst[:, 4:6]
            bk_sb = cst[:, 2:4]
            bq_sb = cst[:, 0:2]
            bpq_sb = cst[:, 6:8]
            rcc = cst[:, 8:16]
            w8k_sb = load_w8("w8k_sb", w8k_d)
            w8q_sb = load_w8("w8q_sb", w8q_d)
            rcb_sb = singles.tile([128, N], BF16)
            nc.sync.dma_start(out=rcb_sb, in_=rcb_d[:, :])
            p8_sb = singles.tile([128, NFP, 2, M], F8, name="p8_sb")
            nc.sync.dma_start(out=p8_sb, in_=p8_d[:, :, :, :])
            w8pq_sb = load_w8("w8pq_sb", w8pq_d)
            xt_a = singles.tile([128, NF, 2 * C], BF16, name="xt_a")
            nc.sync.dma_start(out=xt_a, in_=xT_r[:, :, 0:2 * C])
            wv_sb = singles.tile([128, NF, E], BF16, name="wv_sb")
            nc.sync.dma_start(
                out=wv_sb, in_=wv_d.rearrange("(f p) e -> p f e", p=128))
            bvr_sb = load_small([1, E], BF16, bvr_d, "bvr_sb")
            ones2 = load_small([128, 2], BF16, ones2_d, "ones2")
            x8b_sb = singles.tile([128, NFP, 2, 512], F8, name="x8b_sb")
            nc.sync.dma_start(out=x8b_sb, in_=x8b_d[:, :, :, :])
            xt_b = singles.tile([128, NF, N - 2 * C], BF16, name="xt_b")
            nc.sync.dma_start(out=xt_b, in_=xT_r[:, :, 2 * C:N])
            wo_sb = singles.tile([128, 2, D], BF16)
            nc.sync.dma_start(
                out=wo_sb, in_=wo_d.rearrange("(t p) o -> p t o", p=128))
            ones = singles.tile([1, 128], BF16, name="ones")
            nc.gpsimd.memset(ones, 1.0)

            x8_sb = [x8a_sb, x8b_sb]

            def xt_tok(f, tb):
                # x tile f, token chunk tb, from the early/late split
                if tb < 2:
                    return xt_a[:, f, tb * 128:(tb + 1) * 128]
                return xt_b[:, f, (tb - 2) * 128:(tb - 1) * 128]

            # ---- persistent sbuf tiles ----
            pcT_sb = singles.tile([128, 2, N], ADT)     # [feat, hp, tok], x16
            kT_sb = singles.tile([128, 2, N], ADT)      # x16
            qTrc_sb = singles.tile([128, 2, N], ADT)    # (16q+16bq)*rc*SCALE/256
            bdpq = singles.tile([128, 2, 128], ADT)     # block-diag pq, x16
            nc.vector.memset(bdpq, 0.0)
            vtok_sb = [singles.tile([128, E], ADT, name=f"vtok{t}")
                       for t in range(NCH)]
            # K chunks token-major, via XBAR DMA transpose off the critical
            # path (GPSIMD has no PSUM port; PE transpose + copy would burn
            # DVE/Act time the kernel is short on).
            ktc_sb = singles.tile([128, NCH, 2, 128], ADT, name="ktc_sb")
            # head-zero-padded K for the per-head G matmul: kTp[h] carries
            # head h's 64 feature rows in their native partitions and zeros
            # in the other 64, so contracting all 128 partitions against
            # qTrc (both heads) yields exactly head h's G.
            kTp = [singles.tile([128, 2, N], ADT, name=f"kTp{h}")
                   for h in range(2)]
            nc.gpsimd.memset(kTp[0][64:128, :, :], 0.0)
            nc.gpsimd.memset(kTp[1][0:64, :, :], 0.0)
            # same trick for z^T in pass 2, double-buffered across calls
            at_pad = [[singles.tile([128, 128], ADT, name=f"atp{par}_{h}")
                       for h in range(2)] for par in range(2)]
            for par in range(2):
                nc.vector.memset(at_pad[par][0][64:128, :], 0.0)
                nc.vector.memset(at_pad[par][1][0:64, :], 0.0)
            attnT_sb = [singles.tile([128, 2, C], ADT, name=f"attnT{t}")
                        for t in range(NCH)]
            # merged state copy: [. , (T 0:128 | S 128:256)] per hp
            ST_bd = [singles.tile([128, 256], ADT, name=f"ST{hp}")
                     for hp in range(2)]

            # persistent psum state bank, block-diagonal [128,128] regions,
            # T and S adjacent per hp so one [128,256] copy drains both:
            #   hp0: T [:, 0:128],   S [:, 128:256]
            #   hp1: T [:, 256:384], S [:, 384:512]
            state = psum.tile([128, 512], F32, tag="state", name="state")
            nc.vector.memset(state, 0.0)

            # ---- projections (fp8 DoubleRow), per token-half (nh) ----
            def proj_chain(kind, et, nh):
                w_sb, b_sb, dst = {"pc": (w8pc_sb, bpc_sb, pcT_sb),
                                   "k": (w8k_sb, bk_sb, kT_sb),
                                   "q": (w8q_sb, bq_sb, qTrc_sb)}[kind]
                pp = psum.tile([128, 512], F32, tag="pp", bufs=2, name="ppc")
                for fp in range(NFP):
                    nc.tensor.matmul(
                        pp, w_sb[:, et, fp, :, :],
                        x8_sb[nh][:, fp, :, :],
                        start=(fp == 0), stop=(fp == NFP - 1),
                        perf_mode=PM.DoubleRow)
                if kind == "q":
                    nc.vector.scalar_tensor_tensor(
                        dst[:, et, nh * 512:(nh + 1) * 512], pp,
                        b_sb[:, et:et + 1],
                        rcb_sb[:, nh * 512:(nh + 1) * 512],
                        ALU.add, ALU.mult)
                else:
                    nc.scalar.activation(
                        dst[:, et, nh * 512:(nh + 1) * 512], pp,
                        AF.Identity, bias=b_sb[:, et:et + 1])

            def k_finish(et, nh):
                cols = slice(nh * 512, (nh + 1) * 512)
                # token-major K chunks for the state-S update
                for cc in range(4 * nh, 4 * nh + 4):
                    nc.sync.dma_start(
                        out=ktc_sb[:, cc, et, :],
                        in_=kT_sb[:, et, cc * C:(cc + 1) * C],
                        transpose=True)
                # head-padded copies for the G matmuls (zero halves persist;
                # SBUF-to-SBUF, so the otherwise-idle Pool engine does them)
                nc.gpsimd.tensor_copy(kTp[0][0:64, et, cols],
                                      kT_sb[0:64, et, cols])
                nc.gpsimd.tensor_copy(kTp[1][64:128, et, cols],
                                      kT_sb[64:128, et, cols])

            def proj_half(nh):
                for kind in ("pc", "k", "q"):
                    for et in range(2):
                        proj_chain(kind, et, nh)
                        if kind == "k":
                            k_finish(et, nh)

            def proj_pq():
                for hp in range(2):
                    ppq = psum.tile([128, 512], F32, tag="pp", bufs=2,
                                    name="pppq")
                    for fp in range(NFP):
                        nc.tensor.matmul(
                            ppq[:, 0:M],
                            w8pq_sb[:, hp, fp, :, :],
                            p8_sb[:, fp, :, :],
                            start=(fp == 0), stop=(fp == NFP - 1),
                            perf_mode=PM.DoubleRow)
                    for h in range(2):
                        sl = slice(64 * h, 64 * h + 64)
                        nc.vector.tensor_scalar_add(
                            bdpq[sl, hp, 64 * h:64 * h + 64], ppq[sl, 0:M],
                            bpq_sb[sl, hp:hp + 1])

            def proj_v(tb):
                pkv = psum.tile([128, 512], F32, tag="pp", bufs=2, name="pkv")
                for f in range(NF):
                    nc.tensor.matmul(
                        pkv[:, 0:E], xt_tok(f, tb),
                        wv_sb[:, f, :],
                        start=(f == 0), stop=False)
                nc.tensor.matmul(pkv[:, 0:E], ones, bvr_sb,
                                 start=False, stop=True)
                nc.scalar.activation(vtok_sb[tb], pkv[:, 0:E], AF.Identity)

            # ---- attention ----
            attn_st = {}

            def attn_call(c, hp):
                attn_pass1(c, hp)
                attn_pass2(c, hp)

            def attn_pass1(c, hp):
                tok = slice(c * C, (c + 1) * C)
                par = (2 * c + hp) % 2
                # psum packing
                A = psum.tile([128, 512], F32, tag="pca", bufs=3, name="A")
                Bp = psum.tile([128, 512], F32, tag="pcb", bufs=2, name="Bp")
                pz = A[:, 0:128]
                awT = A[:, 128:256]
                gmp = A[:, 256:512]          # both heads adjacent
                g2p = Bp[:, 0:256]           # both heads adjacent
                pan = Bp[:, 256:384]
                # att lives in B (double-buffered) so the next call's
                # transpose never waits on this call's late rs reciprocal.
                att = Bp[:, 384:448].bitcast(ADT)
                # rs reuses pz's columns: pz is drained by the ez exp long
                # before the rowsums run, and bank A now triple-buffers.
                rs = A[0:1, 0:256]

                # Z_c: pz = pcT^T @ bdpq  -> [tok, m-pair]  (scaled x256)
                nc.tensor.matmul(pz, pcT_sb[:, hp, tok], bdpq[:, hp, :],
                                 start=True, stop=True)
                ez = work.tile([128, 128], ADT, name="ez")
                nc.scalar.activation(ez, pz, AF.Exp, scale=BETA2)
                z = work.tile([128, 128], ADT, name="z")
                nc.scalar.activation(z, ez, AF.Ln, bias=1.0, scale=1.0)

                # Z^T via PE transpose into the head-padded pair (Act does
                # the copies: DVE is the saturated engine in steady state)
                nc.tensor.transpose(att, z, identb)
                nc.scalar.activation(at_pad[par][0][0:64, :], att[0:64, :],
                                     AF.Identity)
                nc.scalar.activation(at_pad[par][1][64:128, :],
                                     att[64:128, :], AF.Identity)

                # G^T = K Q_rc^T per head via the padded-K full contraction,
                # masked with one wide DVE op
                gm = work.tile([128, 256], ADT, name="gm")
                for h in range(2):
                    nc.tensor.matmul(gmp[:, 128 * h:128 * h + 128],
                                     kTp[h][:, hp, tok],
                                     qTrc_sb[:, hp, tok],
                                     start=True, stop=True)
                nc.vector.tensor_tensor(gm, gmp, triu2, ALU.mult)

                # awT[m-pair, tok] = Z^T Gm (+ S^T Q_rc)
                for h in range(2):
                    s = slice(64 * h, 64 * h + 64)
                    nc.tensor.matmul(awT[s, :], z[:, s],
                                     gm[:, 128 * h:128 * h + 128],
                                     start=True, stop=(c == 0),
                                     tile_position=(0, 64 * h))
                if c > 0:
                    nc.tensor.matmul(awT, ST_bd[hp][:, 128:256],
                                     qTrc_sb[:, hp, tok],
                                     start=False, stop=True,
                                     skip_group_check=True)

                # P~^T = exp(awT), unnormalized, directly m-major
                pt = work.tile([128, 128], ADT, name="pt")
                nc.scalar.activation(pt, awT, AF.Exp, scale=1.0)

                # rowsums over m (partition dim) -> [1, 2*128] on partition 0
                for h in range(2):
                    nc.tensor.matmul(rs[:, 128 * h:128 * h + 128],
                                     ones2[:, h:h + 1], pt,
                                     start=True, stop=True,
                                     tile_position=(0, 0))
                rcp = work.tile([1, 256], F32, name="rcp")
                nc.vector.reciprocal(rcp, rs)
                # layered broadcast of 1/rowsum on Pool; rc folds into outproj
                scl = work.tile([128, 128], F32, name="scl")
                nc.gpsimd.partition_broadcast(scl, rcp[:, 128:256],
                                              channels=128)
                nc.gpsimd.partition_broadcast(
                    scl[0:64, :], rcp[:, 0:128], channels=64)

                attn_st[(c, hp)] = (A, Bp, z, pt, scl)

            def attn_pass2(c, hp):
                tok = slice(c * C, (c + 1) * C)
                par = (2 * c + hp) % 2
                A, Bp, z, pt, scl = attn_st.pop((c, hp))
                g2p = Bp[:, 0:256]
                pan = Bp[:, 256:384]
                # pass 2: G2^T = Z P~^T per head via padded z^T, masked wide
                g2m = work.tile([128, 256], ADT, name="g2m")
                for h in range(2):
                    nc.tensor.matmul(g2p[:, 128 * h:128 * h + 128],
                                     at_pad[par][h], pt,
                                     start=True, stop=True)
                nc.vector.tensor_tensor(g2m, g2p, triu2, ALU.mult)

                # attn^T = V^T G2m (+ T^T P~^T), then normalize via scl
                for h in range(2):
                    nc.tensor.matmul(
                        pan[64 * h:64 * h + 64, :],
                        vtok_sb[c][:, hp * 128 + 64 * h:hp * 128 + 64 * h + 64],
                        g2m[:, 128 * h:128 * h + 128],
                        start=True, stop=(c == 0),
                        tile_position=(0, 64 * h))
                if c > 0:
                    nc.tensor.matmul(pan, ST_bd[hp][:, 0:128], pt,
                                     start=False, stop=True,
                                     skip_group_check=True)
                nc.vector.tensor_tensor(attnT_sb[c][:, hp, :], pan, scl,
                                        ALU.mult)

                # ---- state updates (block-diag accumulate in psum) ----
                Tp = state[:, 256 * hp:256 * hp + 128]
                Sp = state[:, 256 * hp + 128:256 * hp + 256]
                for h in range(2):
                    s = slice(64 * h, 64 * h + 64)
                    nc.tensor.matmul(Sp[s, s], ktc_sb[:, c, hp, s], z[:, s],
                                     start=False, stop=True,
                                     skip_group_check=True,
                                     tile_position=(0, 64 * h))
                    nc.tensor.matmul(
                        Tp[s, s], z[:, s],
                        vtok_sb[c][:, hp * 128 + 64 * h:hp * 128 + 64 * h + 64],
                        start=False, stop=True,
                        skip_group_check=True,
                        tile_position=(0, 64 * h))
                if c < NCH - 1:
                    # one merged copy: [T | S] for this hp (Act/DVE alternate)
                    src = state[:, 256 * hp:256 * hp + 256]
                    if hp == 0:
                        nc.scalar.activation(ST_bd[hp], src, AF.Identity)
                    else:
                        nc.vector.tensor_copy(ST_bd[hp], src)

            def out_block(c, ets=(0, 1), final=False):
                tok = slice(c * C, (c + 1) * C)
                for oh in range(2):
                    if ets[0] == 0:
                        out_block.po[oh] = psum.tile([128, 512], F32,
                                                     tag="pp", bufs=2,
                                                     name="po")
                    po = out_block.po[oh]
                    for et in ets:
                        nc.tensor.matmul(
                            po, attnT_sb[c][:, et, :],
                            wo_sb[:, et, oh * 512:(oh + 1) * 512],
                            start=(et == 0), stop=(et == 1))
                    if ets[-1] == 1:
                        def scale_out(dst, src):
                            if oh == 0:
                                nc.vector.tensor_scalar_mul(
                                    dst, src, rcc[:, c:c + 1])
                            else:
                                nc.scalar.activation(dst, src, AF.Identity,
                                                     scale=rcc[:, c:c + 1])
                        if final:
                            # two half-width drains into one buffer, one DMA
                            ob = obuf.tile([128, 512], BF16, name="obf")
                            for q in range(2):
                                cols = slice(256 * q, 256 * q + 256)
                                scale_out(ob[:, cols], po[:, cols])
                            nc.sync.dma_start(
                                out=out_d[tok, oh * 512:(oh + 1) * 512],
                                in_=ob)
                        else:
                            ob = obuf.tile([128, 512], BF16, name="ob")
                            scale_out(ob, po)
                            nc.sync.dma_start(
                                out=out_d[tok, oh * 512:(oh + 1) * 512],
                                in_=ob)
            out_block.po = [None, None]

            proj_half(0)
            proj_pq()
            proj_v(0)

            def chain1(kind, et):
                proj_chain(kind, et, 1)
                if kind == "k":
                    k_finish(et, 1)

            filler = ([lambda tb=tb: proj_v(tb) for tb in range(1, 4)]
                      + [lambda et=et, k=k: chain1(k, et)
                         for k in ("pc", "k", "q") for et in range(2)]
                      + [lambda tb=tb: proj_v(tb) for tb in range(4, NCH)])
            fi = 0
            for c in range(NCH):
                last = c == NCH - 1
                attn_pass1(c, 0)
                attn_pass1(c, 1)
                attn_pass2(c, 0)
                if last:
                    out_block(c, ets=(0,))
                attn_pass2(c, 1)
                # drip-feed remaining projection work between chunks
                for _ in range(2 if c < 4 else 0):
                    if fi < len(filler):
                        filler[fi]()
                        fi += 1
                if last:
                    out_block(c, ets=(1,), final=True)
                else:
                    out_block(c)
            while fi < len(filler):
                filler[fi]()
                fi += 1

    # Patch the act-table map so the load-placement pass only ever picks
    # natural_log_exp_and_others (the one set with Exp+Ln): every other set
    # is emptied, so exactly one LoadActFuncSet is emitted for the whole
    # kernel instead of reloads alternating between sets.
    import concourse.bacc as _bacc_mod
    from concourse.hw_specs import get_activation_tables as _gat
    _orig_gat = _bacc_mod.get_activation_tables

    def _patched_gat(arch):
        t = _gat(arch)
        keep = t.get("natural_log_exp_and_others")
        return {name: (s if s is keep else set())
                for name, s in t.items()}

    _bacc_mod.get_activation_tables = _patched_gat
    try:
        nc.compile()
    finally:
        _bacc_mod.get_activation_tables = _orig_gat
    return nc


_CACHE = {}


def _get_nc():
    import os
    phase = int(os.environ.get("KPHASE", "3"))
    key = f"nc{phase}"
    if key not in _CACHE:
        _CACHE[key] = build_bass(phase)
    return _CACHE[key]


def _pair8(a):
    """[D, X] f32 -> [128, NFP, 2, X] fp8 activation pair layout."""
    import ml_dtypes
    f8 = ml_dtypes.float8_e4m3
    X = a.shape[1]
    return np.ascontiguousarray(
        a.reshape(NFP, 2, 128, X).transpose(2, 0, 1, 3)).astype(f8)


def _pairw8(a):
    """[D, E] f32 -> [128, 2, NFP, 2, 128] fp8 weight pair layout."""
    import ml_dtypes
    f8 = ml_dtypes.float8_e4m3
    return np.ascontiguousarray(
        a.reshape(NFP, 2, 128, 2, 128).transpose(2, 3, 0, 1, 4)).astype(f8)


def make_in_maps(query, p, Wq, bq, Wpq, bpq, Wpc, bpc, Wk, bk, Wv, bv, Wo, bo):
    import ml_dtypes
    bf = ml_dtypes.bfloat16
    f32 = lambda a: np.ascontiguousarray(np.asarray(a), dtype=np.float32)
    query, p = f32(query), f32(p)
    Wq, Wpq, Wpc, Wk, Wv, Wo = map(f32, (Wq, Wpq, Wpc, Wk, Wv, Wo))
    bq, bpq, bpc, bk, bv, bo = map(f32, (bq, bpq, bpc, bk, bv, bo))
    # rc folds: qTrc carries rc*SCALE/256 (both k' and q' are x16)
    rc = (1.0 / ((np.arange(N) + 1.0) * BETA)).astype(np.float32)
    rcb = np.ascontiguousarray(
        np.broadcast_to((rc * SCALE / (WS * WS))[None, :], (128, N)))
    ones2 = np.zeros((128, 2), bf)
    ones2[0:64, 0] = 1
    ones2[64:128, 1] = 1
    rcc = np.ascontiguousarray(rc.reshape(NCH, 128).T)

    def col2(v):  # (256,) -> (128, 2)
        return np.ascontiguousarray(v.reshape(2, 128).T)

    # per-batch fp8/bf16 activations (shared across the 4 cores of a batch)
    xT = [np.ascontiguousarray(query[b].T) for b in range(B)]
    x8a = [_pair8(x[:, 0:512]) for x in xT]
    x8b = [_pair8(x[:, 512:1024]) for x in xT]
    xbf = [x.astype(bf) for x in xT]
    p8 = [_pair8(np.ascontiguousarray(p[b].T)) for b in range(B)]

    in_maps = []
    for core in range(NCORES):
        b = core // 4
        hs = (core % 4) * HPC
        cols = slice(hs * DH, (hs + HPC) * DH)
        cst = np.zeros((128, 16), np.float32)
        cst[:, 0:2] = col2(bq[cols] * WS)
        cst[:, 2:4] = col2(bk[cols] * WS)
        cst[:, 4:6] = col2(bpc[cols] * WS)
        cst[:, 6:8] = col2(bpq[cols] * WS)
        cst[:, 8:16] = rcc
        m = {
            "x8a": x8a[b],
            "x8b": x8b[b],
            "p8": p8[b],
            "xT": xbf[b],
            "w8q": _pairw8(np.ascontiguousarray(Wq[cols, :].T) * WS),
            "w8k": _pairw8(np.ascontiguousarray(Wk[cols, :].T) * WS),
            "w8pc": _pairw8(np.ascontiguousarray(Wpc[cols, :].T) * WS),
            "w8pq": _pairw8(np.ascontiguousarray(Wpq[cols, :].T) * WS),
            "wv": np.ascontiguousarray(Wv[cols, :].T).astype(bf),
            "wo": np.ascontiguousarray(Wo[:, cols].T).astype(bf),
            "cst": cst,
            "bvr": np.ascontiguousarray(bv[cols].reshape(1, E)).astype(bf),
            "rcb": rcb.astype(bf),
            "ones2": ones2,
        }
        in_maps.append(m)
    return in_maps


def kernel(query, p, dec_input_mask=None, p_mask=None,
           Wq=None, bq=None, Wpq=None, bpq=None, Wpc=None, bpc=None,
           Wk=None, bk=None, Wv=None, bv=None, Wo=None, bo=None,
           _trace=False, _trace_kwargs=None):
    in_maps = make_in_maps(query, p, Wq, bq, Wpq, bpq, Wpc, bpc,
                           Wk, bk, Wv, bv, Wo, bo)
    res = run_bass_kernel_spmd(_get_nc(), in_maps, core_ids=list(range(NCORES)),
                               trace=_trace, **(_trace_kwargs or {}))
    bo = np.asarray(bo, dtype=np.float32)
    out = np.zeros((B, N, D), np.float32)
    out += bo.reshape(1, 1, D)
    for core in range(NCORES):
        out[core // 4] += res.results[core]["outp"].astype(np.float32)
    if _trace:
        kernel.last_result = res
    return out


# revision 73
# speedup vs baseline: 1.0172x; 1.0172x over previous
"""Trainium2 Bass kernel for nn_LunaCausalAttention.

Sharding: 8 cores; core c handles batch b = c//4 and heads hs = 4*(c%4) .. hs+4.

v2 over the 82.7us baseline:
- q/k/pc/pq projections run in fp8e4m3 with DoubleRow perf mode (2x contract
  per pass, 0.5 cycles/row): weights are pre-scaled x16 on the host so they
  sit in fp8's normal range; the 1/16 factors fold into the host-side
  constants (rcb, beta2, biases) with no extra device ops. v and the output
  projection stay bf16 (fp8 there costs ~4e-2 end-to-end error).
- The backend compiler rejects programs that mix DoubleRow matmuls with
  sub-128-partition matmul operands, so the per-head G/G2 contractions use
  zero-padded stationaries (kTp / at_pad) and contract the full 128
  partitions; the zero half kills the other head's contribution. Padded
  tiles are persistent: their zero halves are memset once and never
  rewritten.
- K chunks are re-laid token-major by XBAR DMA transpose (SP queue, off the
  critical path) instead of PE transpose + DVE copy.
- Elementwise/copy ops are pinned to engines (GPSIMD has no PSUM port, so
  only SBUF-only work goes to Pool); gm/g2m masks are single wide DVE ops.
- S and T state copies merged into one [128,256] copy per attention call.
- The activation-table patch routes every act-func set to
  natural_log_exp_and_others so exactly one LoadActFuncSet is emitted.
- DMA order: fp8 x/weights first (halved bytes), xT fp8 split per-half so
  the first projection chain starts earlier; last chunk's output is split
  across the two head-pair passes to shorten the tail.
"""
import numpy as np

import concourse.bass as bass
import concourse.mybir as mybir
import concourse.tile as tile
from concourse import bacc
from concourse.masks import make_upper_triangular, make_identity
from concourse.bass_utils import run_bass_kernel_spmd

# static shapes
B, N, D, M, H, DH = 2, 1024, 1024, 64, 16, 64
C = 128                 # token chunk
NCH = N // C            # 8 chunks
NCORES = 8
HPC = 4                 # heads per core
E = HPC * DH            # 256 per-core head features
NF = D // 128           # 8 contraction tiles
NFP = NF // 2           # 4 DoubleRow pair tiles
BETA = float(np.log(2.0))
SCALE = DH ** -0.5
WS = 16.0               # host weight pre-scale for fp8
BETA2 = BETA * SCALE / (WS * WS)   # exp scale for pz (pc' ~ 16pc, pq' ~ 16pq)

F32 = mybir.dt.float32
F16 = mybir.dt.float16
BF16 = mybir.dt.bfloat16
F8 = mybir.dt.float8e4
ADT = BF16              # attention-core operand dtype
AF = mybir.ActivationFunctionType
ALU = mybir.AluOpType
PM = mybir.MatmulPerfMode


def build_bass(phase=3):
    nc = bacc.Bacc(None, target_bir_lowering=False)

    # ---- I/O ----
    # fp8 activation pair layout: [p, fp, t, n] = src[(2*fp+t)*128 + p, n]
    # fp8 weight pair layout: [p, et, fp, t, e] so each (et, fp) slice is a
    # contiguous [128, 2, 128] block (the verified DoubleRow AP shape).
    x8a_d = nc.dram_tensor("x8a", [128, NFP, 2, 512], F8, kind="ExternalInput")
    x8b_d = nc.dram_tensor("x8b", [128, NFP, 2, 512], F8, kind="ExternalInput")
    p8_d = nc.dram_tensor("p8", [128, NFP, 2, M], F8, kind="ExternalInput")
    w8q_d = nc.dram_tensor("w8q", [128, 2, NFP, 2, 128], F8,
                           kind="ExternalInput")
    w8k_d = nc.dram_tensor("w8k", [128, 2, NFP, 2, 128], F8,
                           kind="ExternalInput")
    w8pc_d = nc.dram_tensor("w8pc", [128, 2, NFP, 2, 128], F8,
                            kind="ExternalInput")
    w8pq_d = nc.dram_tensor("w8pq", [128, 2, NFP, 2, 128], F8,
                            kind="ExternalInput")
    xT_d = nc.dram_tensor("xT", [D, N], BF16, kind="ExternalInput")  # for v
    wv_d = nc.dram_tensor("wv", [D, E], BF16, kind="ExternalInput")
    wo_d = nc.dram_tensor("wo", [E, D], BF16, kind="ExternalInput")
    # packed f32 constants: bq 0:2 | bk 2:4 | bpc 4:6 | bpq 6:8 | rcc 8:16
    cst_d = nc.dram_tensor("cst", [128, 16], F32, kind="ExternalInput")
    bvr_d = nc.dram_tensor("bvr", [1, E], BF16, kind="ExternalInput")
    rcb_d = nc.dram_tensor("rcb", [128, N], BF16, kind="ExternalInput")
    ones2_d = nc.dram_tensor("ones2", [128, 2], BF16, kind="ExternalInput")
    out_d = nc.dram_tensor("outp", [N, D], BF16, kind="ExternalOutput")

    with tile.TileContext(nc) as tc:
        with (
            tc.tile_pool(name="singles", bufs=1) as singles,
            tc.tile_pool(name="work", bufs=5) as work,
            tc.tile_pool(name="obuf", bufs=4) as obuf,
            tc.tile_pool(name="psum", bufs=1, space="PSUM") as psum,
        ):
            # ---- constants (device-generated) ----
            triu2 = singles.tile([128, 2 * C], F32)     # two upper-tri copies
            make_upper_triangular(nc, triu2[:, 0:C], val=1.0, diag=True)
            make_upper_triangular(nc, triu2[:, C:2 * C], val=1.0, diag=True)
            identb = singles.tile([128, 128], ADT)
            make_identity(nc, identb)

            # ---- DMA, in compute-need order ----
            def load_small(shape, dt, dram, name):
                t = singles.tile(shape, dt, name=name)
                nc.sync.dma_start(out=t, in_=dram[:, :])
                return t

            def load_w8(name, dram):
                w = singles.tile([128, 2, NFP, 2, 128], F8, name=name)
                nc.sync.dma_start(out=w, in_=dram[:, :, :, :, :])
                return w

            xT_r = xT_d.rearrange("(f p) n -> p f n", p=128)
            x8a_sb = singles.tile([128, NFP, 2, 512], F8, name="x8a_sb")
            nc.sync.dma_start(out=x8a_sb, in_=x8a_d[:, :, :, :])
            w8pc_sb = load_w8("w8pc_sb", w8pc_d)
            cst = load_small([128, 16], F32, cst_d, "cst")
            bpc_sb = cst[:, 4:6]
            bk_sb = cst[:, 2:4]
            bq_sb = cst[:, 0:2]
            bpq_sb = cst[:, 6:8]
            rcc = cst[:, 8:16]
            w8k_sb = load_w8("w8k_sb", w8k_d)
            w8q_sb = load_w8("w8q_sb", w8q_d)
            rcb_sb = singles.tile([128, N], BF16)
            nc.sync.dma_start(out=rcb_sb, in_=rcb_d[:, :])
            p8_sb = singles.tile([128, NFP, 2, M], F8, name="p8_sb")
            nc.sync.dma_start(out=p8_sb, in_=p8_d[:, :, :, :])
            w8pq_sb = load_w8("w8pq_sb", w8pq_d)
            xt_a = singles.tile([128, NF, 2 * C], BF16, name="xt_a")
            nc.sync.dma_start(out=xt_a, in_=xT_r[:, :, 0:2 * C])
            wv_sb = singles.tile([128, NF, E], BF16, name="wv_sb")
            nc.sync.dma_start(
                out=wv_sb, in_=wv_d.rearrange("(f p) e -> p f e", p=128))
            bvr_sb = load_small([1, E], BF16, bvr_d, "bvr_sb")
            ones2 = load_small([128, 2], BF16, ones2_d, "ones2")
            x8b_sb = singles.tile([128, NFP, 2, 512], F8, name="x8b_sb")
            nc.sync.dma_start(out=x8b_sb, in_=x8b_d[:, :, :, :])
            xt_b = singles.tile([128, NF, N - 2 * C], BF16, name="xt_b")
            nc.sync.dma_start(out=xt_b, in_=xT_r[:, :, 2 * C:N])
            wo_sb = singles.tile([128, 2, D], BF16)
            nc.sync.dma_start(
                out=wo_sb, in_=wo_d.rearrange("(t p) o -> p t o", p=128))
            ones = singles.tile([1, 128], BF16, name="ones")
            nc.gpsimd.memset(ones, 1.0)

            x8_sb = [x8a_sb, x8b_sb]

            def xt_tok(f, tb):
                # x tile f, token chunk tb, from the early/late split
                if tb < 2:
                    return xt_a[:, f, tb * 128:(tb + 1) * 128]
                return xt_b[:, f, (tb - 2) * 128:(tb - 1) * 128]

            # ---- persistent sbuf tiles ----
            pcT_sb = singles.tile([128, 2, N], ADT)     # [feat, hp, tok], x16
            kT_sb = singles.tile([128, 2, N], ADT)      # x16
            qTrc_sb = singles.tile([128, 2, N], ADT)    # (16q+16bq)*rc*SCALE/256
            bdpq = singles.tile([128, 2, 128], ADT)     # block-diag pq, x16
            nc.vector.memset(bdpq, 0.0)
            vtok_sb = [singles.tile([128, E], ADT, name=f"vtok{t}")
                       for t in range(NCH)]
            # K chunks token-major, via XBAR DMA transpose off the critical
            # path (GPSIMD has no PSUM port; PE transpose + copy would burn
            # DVE/Act time the kernel is short on).
            ktc_sb = singles.tile([128, NCH, 2, 128], ADT, name="ktc_sb")
            # head-zero-padded K for the per-head G matmul: kTp[h] carries
            # head h's 64 feature rows in their native partitions and zeros
            # in the other 64, so contracting all 128 partitions against
            # qTrc (both heads) yields exactly head h's G.
            kTp = [singles.tile([128, 2, N], ADT, name=f"kTp{h}")
                   for h in range(2)]
            nc.gpsimd.memset(kTp[0][64:128, :, :], 0.0)
            nc.gpsimd.memset(kTp[1][0:64, :, :], 0.0)
            # same trick for z^T in pass 2, double-buffered across calls
            at_pad = [[singles.tile([128, 128], ADT, name=f"atp{par}_{h}")
                       for h in range(2)] for par in range(2)]
            for par in range(2):
                nc.vector.memset(at_pad[par][0][64:128, :], 0.0)
                nc.vector.memset(at_pad[par][1][0:64, :], 0.0)
            attnT_sb = [singles.tile([128, 2, C], ADT, name=f"attnT{t}")
                        for t in range(NCH)]
            # merged state copy: [. , (T 0:128 | S 128:256)] per hp
            ST_bd = [singles.tile([128, 256], ADT, name=f"ST{hp}")
                     for hp in range(2)]

            # persistent psum state bank, block-diagonal [128,128] regions,
            # T and S adjacent per hp so one [128,256] copy drains both:
            #   hp0: T [:, 0:128],   S [:, 128:256]
            #   hp1: T [:, 256:384], S [:, 384:512]
            state = psum.tile([128, 512], F32, tag="state", name="state")
            nc.vector.memset(state, 0.0)

            # ---- projections (fp8 DoubleRow), per token-half (nh) ----
            def proj_chain(kind, et, nh):
                w_sb, b_sb, dst = {"pc": (w8pc_sb, bpc_sb, pcT_sb),
                                   "k": (w8k_sb, bk_sb, kT_sb),
                                   "q": (w8q_sb, bq_sb, qTrc_sb)}[kind]
                pp = psum.tile([128, 512], F32, tag="pp", bufs=2, name="ppc")
                for fp in range(NFP):
                    nc.tensor.matmul(
                        pp, w_sb[:, et, fp, :, :],
                        x8_sb[nh][:, fp, :, :],
                        start=(fp == 0), stop=(fp == NFP - 1),
                        perf_mode=PM.DoubleRow)
                if kind == "q":
                    nc.vector.scalar_tensor_tensor(
                        dst[:, et, nh * 512:(nh + 1) * 512], pp,
                        b_sb[:, et:et + 1],
                        rcb_sb[:, nh * 512:(nh + 1) * 512],
                        ALU.add, ALU.mult)
                else:
                    nc.scalar.activation(
                        dst[:, et, nh * 512:(nh + 1) * 512], pp,
                        AF.Identity, bias=b_sb[:, et:et + 1])

            def k_finish(et, nh):
                cols = slice(nh * 512, (nh + 1) * 512)
                # token-major K chunks for the state-S update
                for cc in range(4 * nh, 4 * nh + 4):
                    nc.sync.dma_start(
                        out=ktc_sb[:, cc, et, :],
                        in_=kT_sb[:, et, cc * C:(cc + 1) * C],
                        transpose=True)
                # head-padded copies for the G matmuls (zero halves persist;
                # SBUF-to-SBUF, so the otherwise-idle Pool engine does them)
                nc.gpsimd.tensor_copy(kTp[0][0:64, et, cols],
                                      kT_sb[0:64, et, cols])
                nc.gpsimd.tensor_copy(kTp[1][64:128, et, cols],
                                      kT_sb[64:128, et, cols])

            def proj_half(nh):
                for kind in ("pc", "k", "q"):
                    for et in range(2):
                        proj_chain(kind, et, nh)
                        if kind == "k":
                            k_finish(et, nh)

            def proj_pq():
                for hp in range(2):
                    ppq = psum.tile([128, 512], F32, tag="pp", bufs=2,
                                    name="pppq")
                    for fp in range(NFP):
                        nc.tensor.matmul(
                            ppq[:, 0:M],
                            w8pq_sb[:, hp, fp, :, :],
                            p8_sb[:, fp, :, :],
                            start=(fp == 0), stop=(fp == NFP - 1),
                            perf_mode=PM.DoubleRow)
                    for h in range(2):
                        sl = slice(64 * h, 64 * h + 64)
                        nc.vector.tensor_scalar_add(
                            bdpq[sl, hp, 64 * h:64 * h + 64], ppq[sl, 0:M],
                            bpq_sb[sl, hp:hp + 1])

            def proj_v(tb):
                pkv = psum.tile([128, 512], F32, tag="pp", bufs=2, name="pkv")
                for f in range(NF):
                    nc.tensor.matmul(
                        pkv[:, 0:E], xt_tok(f, tb),
                        wv_sb[:, f, :],
                        start=(f == 0), stop=False)
                nc.tensor.matmul(pkv[:, 0:E], ones, bvr_sb,
                                 start=False, stop=True)
                nc.scalar.activation(vtok_sb[tb], pkv[:, 0:E], AF.Identity)

            # ---- attention ----
            attn_st = {}

            def attn_call(c, hp):
                attn_pass1(c, hp)
                attn_pass2(c, hp)

            def attn_pass1(c, hp):
                tok = slice(c * C, (c + 1) * C)
                par = (2 * c + hp) % 2
                # psum packing
                A = psum.tile([128, 512], F32, tag="pca", bufs=3, name="A")
                Bp = psum.tile([128, 512], F32, tag="pcb", bufs=2, name="Bp")
                pz = A[:, 0:128]
                awT = A[:, 128:256]
                gmp = A[:, 256:512]          # both heads adjacent
                g2p = Bp[:, 0:256]           # both heads adjacent
                pan = Bp[:, 256:384]
                # att lives in B (double-buffered) so the next call's
                # transpose never waits on this call's late rs reciprocal.
                att = Bp[:, 384:448].bitcast(ADT)
                # rs reuses pz's columns: pz is drained by the ez exp long
                # before the rowsums run, and bank A now triple-buffers.
                rs = A[0:1, 0:256]

                # Z_c: pz = pcT^T @ bdpq  -> [tok, m-pair]  (scaled x256)
                nc.tensor.matmul(pz, pcT_sb[:, hp, tok], bdpq[:, hp, :],
                                 start=True, stop=True)
                ez = work.tile([128, 128], ADT, name="ez")
                nc.scalar.activation(ez, pz, AF.Exp, scale=BETA2)
                z = work.tile([128, 128], ADT, name="z")
                nc.scalar.activation(z, ez, AF.Ln, bias=1.0, scale=1.0)

                # Z^T via PE transpose into the head-padded pair (Act does
                # the copies: DVE is the saturated engine in steady state)
                nc.tensor.transpose(att, z, identb)
                nc.scalar.activation(at_pad[par][0][0:64, :], att[0:64, :],
                                     AF.Identity)
                nc.scalar.activation(at_pad[par][1][64:128, :],
                                     att[64:128, :], AF.Identity)

                # G^T = K Q_rc^T per head via the padded-K full contraction,
                # masked with one wide DVE op
                gm = work.tile([128, 256], ADT, name="gm")
                for h in range(2):
                    nc.tensor.matmul(gmp[:, 128 * h:128 * h + 128],
                                     kTp[h][:, hp, tok],
                                     qTrc_sb[:, hp, tok],
                                     start=True, stop=True)
                nc.vector.tensor_tensor(gm, gmp, triu2, ALU.mult)

                # awT[m-pair, tok] = Z^T Gm (+ S^T Q_rc)
                for h in range(2):
                    s = slice(64 * h, 64 * h + 64)
                    nc.tensor.matmul(awT[s, :], z[:, s],
                                     gm[:, 128 * h:128 * h + 128],
                                     start=True, stop=(c == 0),
                                     tile_position=(0, 64 * h))
                if c > 0:
                    nc.tensor.matmul(awT, ST_bd[hp][:, 128:256],
                                     qTrc_sb[:, hp, tok],
                                     start=False, stop=True,
                                     skip_group_check=True)

                # P~^T = exp(awT), unnormalized, directly m-major
                pt = work.tile([128, 128], ADT, name="pt")
                nc.scalar.activation(pt, awT, AF.Exp, scale=1.0)

                # rowsums over m (partition dim) -> [1, 2*128] on partition 0
                for h in range(2):
                    nc.tensor.matmul(rs[:, 128 * h:128 * h + 128],
                                     ones2[:, h:h + 1], pt,
                                     start=True, stop=True,
                                     tile_position=(0, 0))
                rcp = work.tile([1, 256], F32, name="rcp")
                nc.vector.reciprocal(rcp, rs)
                # layered broadcast of 1/rowsum on Pool; rc folds into outproj
                scl = work.tile([128, 128], F32, name="scl")
                nc.gpsimd.partition_broadcast(scl, rcp[:, 128:256],
                                              channels=128)
                nc.gpsimd.partition_broadcast(
                    scl[0:64, :], rcp[:, 0:128], channels=64)

                attn_st[(c, hp)] = (A, Bp, z, pt, scl)

            def attn_pass2(c, hp):
                tok = slice(c * C, (c + 1) * C)
                par = (2 * c + hp) % 2
                A, Bp, z, pt, scl = attn_st.pop((c, hp))
                g2p = Bp[:, 0:256]
                pan = Bp[:, 256:384]
                # pass 2: G2^T = Z P~^T per head via padded z^T, masked wide
                g2m = work.tile([128, 256], ADT, name="g2m")
                for h in range(2):
                    nc.tensor.matmul(g2p[:, 128 * h:128 * h + 128],
                                     at_pad[par][h], pt,
                                     start=True, stop=True)
                nc.vector.tensor_tensor(g2m, g2p, triu2, ALU.mult)

                # attn^T = V^T G2m (+ T^T P~^T), then normalize via scl
                for h in range(2):
                    nc.tensor.matmul(
                        pan[64 * h:64 * h + 64, :],
                        vtok_sb[c][:, hp * 128 + 64 * h:hp * 128 + 64 * h + 64],
                        g2m[:, 128 * h:128 * h + 128],
                        start=True, stop=(c == 0),
                        tile_position=(0, 64 * h))
                if c > 0:
                    nc.tensor.matmul(pan, ST_bd[hp][:, 0:128], pt,
                                     start=False, stop=True,
                                     skip_group_check=True)
                nc.vector.tensor_tensor(attnT_sb[c][:, hp, :], pan, scl,
                                        ALU.mult)

                # ---- state updates (block-diag accumulate in psum) ----
                Tp = state[:, 256 * hp:256 * hp + 128]
                Sp = state[:, 256 * hp + 128:256 * hp + 256]
                for h in range(2):
                    s = slice(64 * h, 64 * h + 64)
                    nc.tensor.matmul(Sp[s, s], ktc_sb[:, c, hp, s], z[:, s],
                                     start=False, stop=True,
                                     skip_group_check=True,
                                     tile_position=(0, 64 * h))
                    nc.tensor.matmul(
                        Tp[s, s], z[:, s],
                        vtok_sb[c][:, hp * 128 + 64 * h:hp * 128 + 64 * h + 64],
                        start=False, stop=True,
                        skip_group_check=True,
                        tile_position=(0, 64 * h))
                if c < NCH - 1:
                    # one merged copy: [T | S] for this hp (Act/DVE alternate)
                    src = state[:, 256 * hp:256 * hp + 256]
                    if hp == 0:
                        nc.scalar.activation(ST_bd[hp], src, AF.Identity)
                    else:
                        nc.vector.tensor_copy(ST_bd[hp], src)

            def out_block(c, ets=(0, 1), final=False):
                tok = slice(c * C, (c + 1) * C)
                for oh in range(2):
                    if ets[0] == 0:
                        out_block.po[oh] = psum.tile([128, 512], F32,
                                                     tag="pp", bufs=2,
                                                     name="po")
                    po = out_block.po[oh]
                    for et in ets:
                        nc.tensor.matmul(
                            po, attnT_sb[c][:, et, :],
                            wo_sb[:, et, oh * 512:(oh + 1) * 512],
                            start=(et == 0), stop=(et == 1))
                    if ets[-1] == 1:
                        def scale_out(dst, src):
                            if oh == 0:
                                nc.vector.tensor_scalar_mul(
                                    dst, src, rcc[:, c:c + 1])
                            else:
                                nc.scalar.activation(dst, src, AF.Identity,
                                                     scale=rcc[:, c:c + 1])
                        if final:
                            # two half-width drains into one buffer, one DMA
                            ob = obuf.tile([128, 512], BF16, name="obf")
                            for q in range(2):
                                cols = slice(256 * q, 256 * q + 256)
                                scale_out(ob[:, cols], po[:, cols])
                            nc.sync.dma_start(
                                out=out_d[tok, oh * 512:(oh + 1) * 512],
                                in_=ob)
                        else:
                            ob = obuf.tile([128, 512], BF16, name="ob")
                            scale_out(ob, po)
                            nc.sync.dma_start(
                                out=out_d[tok, oh * 512:(oh + 1) * 512],
                                in_=ob)
            out_block.po = [None, None]

            proj_half(0)
            proj_pq()
            proj_v(0)

            def chain1(kind, et):
                proj_chain(kind, et, 1)
                if kind == "k":
                    k_finish(et, 1)

            filler = ([lambda tb=tb: proj_v(tb) for tb in range(1, 4)]
                      + [lambda et=et, k=k: chain1(k, et)
                         for k in ("pc", "k", "q") for et in range(2)]
                      + [lambda tb=tb: proj_v(tb) for tb in range(4, NCH)])
            fi = 0
            for c in range(NCH):
                last = c == NCH - 1
                attn_pass1(c, 0)
                # drip-feed projection work; all 13 filler items must be
                # emitted before chunk 4 reads their outputs (16 slots)
                for _ in range(2 if c < 4 else 0):
                    if fi < len(filler):
                        filler[fi]()
                        fi += 1
                attn_pass1(c, 1)
                attn_pass2(c, 0)
                if last:
                    out_block(c, ets=(0,))
                for _ in range(2 if c < 4 else 0):
                    if fi < len(filler):
                        filler[fi]()
                        fi += 1
                attn_pass2(c, 1)
                if last:
                    out_block(c, ets=(1,), final=True)
                else:
                    out_block(c)
            while fi < len(filler):
                filler[fi]()
                fi += 1

    # Patch the act-table map so the load-placement pass only ever picks
    # natural_log_exp_and_others (the one set with Exp+Ln): every other set
    # is emptied, so exactly one LoadActFuncSet is emitted for the whole
    # kernel instead of reloads alternating between sets.
    import concourse.bacc as _bacc_mod
    from concourse.hw_specs import get_activation_tables as _gat
    _orig_gat = _bacc_mod.get_activation_tables

    def _patched_gat(arch):
        t = _gat(arch)
        keep = t.get("natural_log_exp_and_others")
        return {name: (s if s is keep else set())
                for name, s in t.items()}

    _bacc_mod.get_activation_tables = _patched_gat
    try:
        nc.compile()
    finally:
        _bacc_mod.get_activation_tables = _orig_gat
    return nc


_CACHE = {}


def _get_nc():
    import os
    phase = int(os.environ.get("KPHASE", "3"))
    key = f"nc{phase}"
    if key not in _CACHE:
        _CACHE[key] = build_bass(phase)
    return _CACHE[key]


def _pair8(a):
    """[D, X] f32 -> [128, NFP, 2, X] fp8 activation pair layout."""
    import ml_dtypes
    f8 = ml_dtypes.float8_e4m3
    X = a.shape[1]
    return np.ascontiguousarray(
        a.reshape(NFP, 2, 128, X).transpose(2, 0, 1, 3)).astype(f8)


def _pairw8(a):
    """[D, E] f32 -> [128, 2, NFP, 2, 128] fp8 weight pair layout."""
    import ml_dtypes
    f8 = ml_dtypes.float8_e4m3
    return np.ascontiguousarray(
        a.reshape(NFP, 2, 128, 2, 128).transpose(2, 3, 0, 1, 4)).astype(f8)


def make_in_maps(query, p, Wq, bq, Wpq, bpq, Wpc, bpc, Wk, bk, Wv, bv, Wo, bo):
    import ml_dtypes
    bf = ml_dtypes.bfloat16
    f32 = lambda a: np.ascontiguousarray(np.asarray(a), dtype=np.float32)
    query, p = f32(query), f32(p)
    Wq, Wpq, Wpc, Wk, Wv, Wo = map(f32, (Wq, Wpq, Wpc, Wk, Wv, Wo))
    bq, bpq, bpc, bk, bv, bo = map(f32, (bq, bpq, bpc, bk, bv, bo))
    # rc folds: qTrc carries rc*SCALE/256 (both k' and q' are x16)
    rc = (1.0 / ((np.arange(N) + 1.0) * BETA)).astype(np.float32)
    rcb = np.ascontiguousarray(
        np.broadcast_to((rc * SCALE / (WS * WS))[None, :], (128, N)))
    ones2 = np.zeros((128, 2), bf)
    ones2[0:64, 0] = 1
    ones2[64:128, 1] = 1
    rcc = np.ascontiguousarray(rc.reshape(NCH, 128).T)

    def col2(v):  # (256,) -> (128, 2)
        return np.ascontiguousarray(v.reshape(2, 128).T)

    # per-batch fp8/bf16 activations (shared across the 4 cores of a batch)
    xT = [np.ascontiguousarray(query[b].T) for b in range(B)]
    x8a = [_pair8(x[:, 0:512]) for x in xT]
    x8b = [_pair8(x[:, 512:1024]) for x in xT]
    xbf = [x.astype(bf) for x in xT]
    p8 = [_pair8(np.ascontiguousarray(p[b].T)) for b in range(B)]

    in_maps = []
    for core in range(NCORES):
        b = core // 4
        hs = (core % 4) * HPC
        cols = slice(hs * DH, (hs + HPC) * DH)
        cst = np.zeros((128, 16), np.float32)
        cst[:, 0:2] = col2(bq[cols] * WS)
        cst[:, 2:4] = col2(bk[cols] * WS)
        cst[:, 4:6] = col2(bpc[cols] * WS)
        cst[:, 6:8] = col2(bpq[cols] * WS)
        cst[:, 8:16] = rcc
        m = {
            "x8a": x8a[b],
            "x8b": x8b[b],
            "p8": p8[b],
            "xT": xbf[b],
            "w8q": _pairw8(np.ascontiguousarray(Wq[cols, :].T) * WS),
            "w8k": _pairw8(np.ascontiguousarray(Wk[cols, :].T) * WS),
            "w8pc": _pairw8(np.ascontiguousarray(Wpc[cols, :].T) * WS),
            "w8pq": _pairw8(np.ascontiguousarray(Wpq[cols, :].T) * WS),
            "wv": np.ascontiguousarray(Wv[cols, :].T).astype(bf),
            "wo": np.ascontiguousarray(Wo[:, cols].T).astype(bf),
            "cst": cst,
            "bvr": np.ascontiguousarray(bv[cols].reshape(1, E)).astype(bf),
            "rcb": rcb.astype(bf),
            "ones2": ones2,
        }
        in_maps.append(m)
    return in_maps


def kernel(query, p, dec_input_mask=None, p_mask=None,
           Wq=None, bq=None, Wpq=None, bpq=None, Wpc=None, bpc=None,
           Wk=None, bk=None, Wv=None, bv=None, Wo=None, bo=None,
           _trace=False, _trace_kwargs=None):
    in_maps = make_in_maps(query, p, Wq, bq, Wpq, bpq, Wpc, bpc,
                           Wk, bk, Wv, bv, Wo, bo)
    res = run_bass_kernel_spmd(_get_nc(), in_maps, core_ids=list(range(NCORES)),
                               trace=_trace, **(_trace_kwargs or {}))
    bo = np.asarray(bo, dtype=np.float32)
    out = np.zeros((B, N, D), np.float32)
    out += bo.reshape(1, 1, D)
    for core in range(NCORES):
        out[core // 4] += res.results[core]["outp"].astype(np.float32)
    if _trace:
        kernel.last_result = res
    return out
